# revision 6
# baseline (speedup 1.0000x reference)
import sys
import numpy as np
import ml_dtypes

sys.path.insert(0, "/opt/trn_rl_repo")
sys.path.insert(0, "/opt/trn_rl_repo/concourse")

import concourse.bass as bass
import concourse.bacc as bacc
import concourse.tile as tile
import concourse.mybir as mybir
from concourse.bass_utils import run_bass_kernel_spmd
from concourse.masks import make_identity

BF16 = mybir.dt.bfloat16
F32 = mybir.dt.float32
I32 = mybir.dt.int32
bf16 = ml_dtypes.bfloat16

NCORES = 8
A, B, NB = 100000, 200000, 6
BOND_F, ATOM_F, H, RO = 147, 133, 300, 512
DEPTH = 6
APM = 25  # atoms per mol
BK = B // NCORES  # 25000 bonds/core
AK = A // NCORES  # 12500 atoms/core
MK = AK // APM  # 500 mols/core
BT = 196  # bond tiles of 128 (196*128 = 25088 >= 25000)
BKP = BT * 128  # padded bonds per core
AT = 100  # atom tiles of 125 (100*125 = 12500)
ZROWS = NCORES * BKP
UNROLL = 4

_NC_CACHE = None


def _emit_bond_tail(nc, sb, ps, ident, wh_c, zloc_v, ti, msg_t):
    """Transpose msg tile and matmul against weight chunks; write Z row tile."""
    zp = ps.tile([128, H], F32, name="zp", tag="zp")
    for c, (c0, cw) in enumerate(((0, 128), (128, 128), (256, 44))):
        tp = ps.tile([128, 128], BF16, name="tp", tag="tp")
        nc.tensor.transpose(tp[:cw, :], msg_t[:, c0 : c0 + cw], ident[:])
        mT = sb.tile([128, 128], BF16, name="mT", tag="mT")
        nc.vector.tensor_copy(mT[:cw, :], tp[:cw, :])
        nc.tensor.matmul(
            zp[:], mT[:cw, :], wh_c[c], start=(c == 0), stop=(c == 2)
        )
    zs = sb.tile([128, H], BF16, name="zs", tag="zs")
    nc.vector.tensor_copy(zs[:], zp[:])
    nc.sync.dma_start(out=zloc_v[ti], in_=zs[:])


def build():
    nc = bacc.Bacc("TRN2", target_bir_lowering=False, debug=False, num_devices=NCORES)

    # ---------------- inputs ----------------
    fbT = nc.dram_tensor("fbT", [BOND_F, BT, 128], BF16, kind="ExternalInput")
    faT = nc.dram_tensor("faT", [ATOM_F, AT, 125], BF16, kind="ExternalInput")
    it_idx = nc.dram_tensor("it_idx", [BT, 128, 7], I32, kind="ExternalInput")
    fin_idx = nc.dram_tensor("fin_idx", [AT, 125, 6], I32, kind="ExternalInput")
    wi = nc.dram_tensor("wi", [BOND_F, H], BF16, kind="ExternalInput")
    wh = nc.dram_tensor("wh", [H, H], BF16, kind="ExternalInput")
    wo1 = nc.dram_tensor("wo1", [ATOM_F, H], BF16, kind="ExternalInput")
    wo2 = nc.dram_tensor("wo2", [H, H], BF16, kind="ExternalInput")
    wfc0 = nc.dram_tensor("wfc0", [H, RO], BF16, kind="ExternalInput")
    wfc1 = nc.dram_tensor("wfc1", [RO, RO], BF16, kind="ExternalInput")
    wlast = nc.dram_tensor("wlast", [RO, 1], BF16, kind="ExternalInput")
    bi_r = nc.dram_tensor("bi_r", [128, H], F32, kind="ExternalInput")
    bh_r = nc.dram_tensor("bh_r", [128, H], F32, kind="ExternalInput")
    bo_r = nc.dram_tensor("bo_r", [128, H], F32, kind="ExternalInput")
    bfc0 = nc.dram_tensor("bfc0", [RO, 1], F32, kind="ExternalInput")
    bfc1 = nc.dram_tensor("bfc1", [RO, 1], F32, kind="ExternalInput")
    blast = nc.dram_tensor("blast", [1, 1], F32, kind="ExternalInput")
    psel_in = nc.dram_tensor("psel_in", [125, 5], BF16, kind="ExternalInput")
    out_ext = nc.dram_tensor("out", [1, MK], F32, kind="ExternalOutput")

    with tile.TileContext(nc) as tc:
        with (
            tc.tile_pool(name="const", bufs=1) as cst,
            tc.tile_pool(name="dram", bufs=1, space="DRAM") as dram,
        ):
            # big DRAM buffers
            zbufs = [
                dram.tile([ZROWS, H], BF16, addr_space="Shared", name=f"zfull{t}")
                for t in range(DEPTH)
            ]
            zloc = dram.tile([BKP, H], BF16, name="zloc")
            inpb_d = dram.tile([BT, 128, H], BF16, name="inpb_d")
            mv_d = dram.tile([AT, 5, H], F32, name="mv_d")
            zloc_v = zloc[:].rearrange("(t p) h -> t p h", p=128)

            # resident constants
            ident = cst.tile([128, 128], BF16, name="ident")
            make_identity(nc, ident[:])
            wi_c0 = cst.tile([128, H], BF16, name="wi_c0")
            wi_c1 = cst.tile([19, H], BF16, name="wi_c1", padded_shape=[128, None])
            nc.sync.dma_start(out=wi_c0[:], in_=wi[0:128, :])
            nc.sync.dma_start(out=wi_c1[:], in_=wi[128:BOND_F, :])
            wh_c, wo2_c = [], []
            for c, (c0, cw) in enumerate(((0, 128), (128, 128), (256, 44))):
                t1 = cst.tile([cw, H], BF16, name=f"wh_c{c}", padded_shape=[128, None])
                nc.sync.dma_start(out=t1[:], in_=wh[c0 : c0 + cw, :])
                wh_c.append(t1[:])
                t2 = cst.tile([cw, H], BF16, name=f"wo2_c{c}", padded_shape=[128, None])
                nc.sync.dma_start(out=t2[:], in_=wo2[c0 : c0 + cw, :])
                wo2_c.append(t2[:])
            wo1_c0 = cst.tile([128, H], BF16, name="wo1_c0")
            wo1_c1 = cst.tile([5, H], BF16, name="wo1_c1", padded_shape=[128, None])
            nc.sync.dma_start(out=wo1_c0[:], in_=wo1[0:128, :])
            nc.sync.dma_start(out=wo1_c1[:], in_=wo1[128:ATOM_F, :])
            bi_t = cst.tile([128, H], F32, name="bi_t")
            bh_t = cst.tile([128, H], F32, name="bh_t")
            bo_t = cst.tile([128, H], F32, name="bo_t")
            nc.sync.dma_start(out=bi_t[:], in_=bi_r[:])
            nc.sync.dma_start(out=bh_t[:], in_=bh_r[:])
            nc.sync.dma_start(out=bo_t[:], in_=bo_r[:])
            psel = cst.tile([125, 5], BF16, name="psel")
            nc.sync.dma_start(out=psel[:], in_=psel_in[:])

            rg = [list(range(NCORES))]

            # ---------------- stage 0: inp/msg0/Z0 ----------------
            with (
                tc.tile_pool(name="s0sb", bufs=3) as sb,
                tc.tile_pool(name="s0ps", bufs=2, space="PSUM") as ps,
            ):
                with tc.For_i(0, BT, UNROLL) as i0:
                    for u in range(UNROLL):
                        ti = i0 + u
                        fb0 = sb.tile([128, 128], BF16, name="fb0", tag="fb0")
                        fb1 = sb.tile(
                            [19, 128], BF16, name="fb1", tag="fb1",
                            padded_shape=[128, None],
                        )
                        nc.sync.dma_start(out=fb0[:], in_=fbT[0:128, ti, :])
                        nc.sync.dma_start(out=fb1[:], in_=fbT[128:BOND_F, ti, :])
                        ip = ps.tile([128, H], F32, name="ip", tag="ip")
                        nc.tensor.matmul(ip[:], fb0[:], wi_c0[:], start=True, stop=False)
                        nc.tensor.matmul(
                            ip[:], fb1[:], wi_c1[:19, :], start=False, stop=True
                        )
                        t0 = sb.tile([128, H], F32, name="t0", tag="t0")
                        nc.vector.tensor_tensor(
                            out=t0[:], in0=ip[:], in1=bi_t[:], op=mybir.AluOpType.add
                        )
                        msg_t = sb.tile([128, H], BF16, name="msg_t", tag="msg_t")
                        nc.scalar.activation(
                            msg_t[:], t0[:], mybir.ActivationFunctionType.Relu
                        )
                        inb = sb.tile([128, H], BF16, name="inb", tag="inb")
                        nc.vector.tensor_tensor(
                            out=inb[:], in0=t0[:], in1=bh_t[:], op=mybir.AluOpType.add
                        )
                        nc.sync.dma_start(out=inpb_d[ti], in_=inb[:])
                        _emit_bond_tail(nc, sb, ps, ident, wh_c, zloc_v, ti, msg_t)
            nc.gpsimd.collective_compute(
                "AllGather", mybir.AluOpType.bypass, replica_groups=rg,
                ins=[zloc[:].opt()], outs=[zbufs[0][:].opt()],
            )

            # ---------------- iterations 1..5 ----------------
            srcs = [zbufs[t] for t in range(DEPTH - 1)]
            dsts = [zbufs[t + 1] for t in range(DEPTH - 1)]
            for it in range(DEPTH - 1):
                last = it == DEPTH - 2
                w_chunks = wo2_c if last else wh_c
                src = srcs[it]
                with (
                    tc.tile_pool(name=f"i{it}sb", bufs=3) as sb,
                    tc.tile_pool(name=f"i{it}ps", bufs=2, space="PSUM") as ps,
                ):
                    with tc.For_i(0, BT, UNROLL) as iv:
                        for u in range(UNROLL):
                            ti = iv + u
                            ix = sb.tile([128, 7], I32, name="ix", tag="ix")
                            nc.sync.dma_start(out=ix[:], in_=it_idx[ti])
                            g = sb.tile([128, 7, H], BF16, name="g", tag="g")
                            for j in range(7):
                                nc.gpsimd.indirect_dma_start(
                                    out=g[:, j, :],
                                    out_offset=None,
                                    in_=src[:],
                                    in_offset=bass.IndirectOffsetOnAxis(
                                        ap=ix[:, j : j + 1], axis=0
                                    ),
                                )
                            acc = sb.tile([128, H], F32, name="acc", tag="acc")
                            nc.vector.reduce_sum(
                                acc[:],
                                g[:, 0:6, :].rearrange("p j h -> p h j"),
                                axis=mybir.AxisListType.X,
                            )
                            nc.vector.tensor_tensor(
                                out=acc[:], in0=acc[:], in1=g[:, 6, :],
                                op=mybir.AluOpType.subtract,
                            )
                            inb = sb.tile([128, H], BF16, name="inb", tag="inb")
                            nc.sync.dma_start(out=inb[:], in_=inpb_d[ti])
                            nc.vector.tensor_tensor(
                                out=acc[:], in0=acc[:], in1=inb[:],
                                op=mybir.AluOpType.add,
                            )
                            msg_t = sb.tile([128, H], BF16, name="msg_t", tag="msg_t")
                            nc.scalar.activation(
                                msg_t[:], acc[:], mybir.ActivationFunctionType.Relu
                            )
                            _emit_bond_tail(nc, sb, ps, ident, w_chunks, zloc_v, ti, msg_t)
                nc.gpsimd.collective_compute(
                    "AllGather", mybir.AluOpType.bypass, replica_groups=rg,
                    ins=[zloc[:].opt()], outs=[dsts[it][:].opt()],
                )

            # ---------------- final atom stage ----------------
            zo = dsts[DEPTH - 2]  # gathered ZO table
            with (
                tc.tile_pool(name="fsb", bufs=3) as sb,
                tc.tile_pool(name="fps", bufs=2, space="PSUM") as ps,
            ):
                with tc.For_i(0, AT, UNROLL) as fv:
                    for u in range(UNROLL):
                        ti = fv + u
                        ix = sb.tile([125, 6], I32, name="fix", tag="fix")
                        nc.sync.dma_start(out=ix[:], in_=fin_idx[ti])
                        g = sb.tile([125, 6, H], BF16, name="fg", tag="fg")
                        for j in range(6):
                            nc.gpsimd.indirect_dma_start(
                                out=g[:, j, :],
                                out_offset=None,
                                in_=zo[:],
                                in_offset=bass.IndirectOffsetOnAxis(
                                    ap=ix[:, j : j + 1], axis=0
                                ),
                            )
                        acc = sb.tile([125, H], F32, name="facc", tag="facc")
                        nc.vector.reduce_sum(
                            acc[:],
                            g[:].rearrange("p j h -> p h j"),
                            axis=mybir.AxisListType.X,
                        )
                        fa0 = sb.tile([128, 125], BF16, name="fa0", tag="fa0")
                        fa1 = sb.tile(
                            [5, 125], BF16, name="fa1", tag="fa1",
                            padded_shape=[128, None],
                        )
                        nc.sync.dma_start(out=fa0[:], in_=faT[0:128, ti, :])
                        nc.sync.dma_start(out=fa1[:], in_=faT[128:ATOM_F, ti, :])
                        ap_ = ps.tile([125, H], F32, name="ap_", tag="ap_")
                        nc.tensor.matmul(ap_[:], fa0[:, :], wo1_c0[:], start=True, stop=False)
                        nc.tensor.matmul(ap_[:], fa1[:5, :], wo1_c1[:5, :], start=False, stop=True)
                        nc.vector.tensor_tensor(
                            out=acc[:], in0=acc[:], in1=ap_[:], op=mybir.AluOpType.add
                        )
                        nc.vector.tensor_tensor(
                            out=acc[:], in0=acc[:], in1=bo_t[:125, :], op=mybir.AluOpType.add
                        )
                        ah = sb.tile([125, H], BF16, name="ah", tag="ah")
                        nc.scalar.activation(
                            ah[:], acc[:], mybir.ActivationFunctionType.Relu
                        )
                        mvp = ps.tile([5, H], F32, name="mvp", tag="mvp")
                        nc.tensor.matmul(mvp[:], psel[:], ah[:], start=True, stop=True)
                        mvs = sb.tile([5, H], F32, name="mvs", tag="mvs")
                        nc.vector.tensor_copy(mvs[:], mvp[:])
                        nc.sync.dma_start(out=mv_d[ti], in_=mvs[:])

            # ---------------- readout (static) ----------------
            with (
                tc.tile_pool(name="rsb", bufs=1) as sb,
                tc.tile_pool(name="rps", bufs=1, space="PSUM") as ps,
            ):
                # build mvT [300, 500] as 3 sbuf tiles [cw, 500], scaled by 1/APM
                mt = []
                for c, (c0, cw) in enumerate(((0, 128), (128, 128), (256, 44))):
                    t = sb.tile([cw, MK], BF16, name=f"mt{c}", padded_shape=[128, None])
                    mt.append(t)
                for q in range(4):
                    mvq = sb.tile([125, H], F32, name=f"mvq{q}")
                    nc.sync.dma_start(
                        out=mvq[:],
                        in_=mv_d[:].rearrange("t f h -> (t f) h")[
                            q * 125 : (q + 1) * 125, :
                        ],
                    )
                    mvqb = sb.tile([125, H], BF16, name=f"mvqb{q}")
                    nc.vector.tensor_copy(mvqb[:], mvq[:])
                    for c, (c0, cw) in enumerate(((0, 128), (128, 128), (256, 44))):
                        tp = ps.tile([128, 125], BF16, name="rtp", tag="rtp")
                        nc.tensor.transpose(
                            tp[:cw, :], mvqb[:, c0 : c0 + cw], ident[:125, :125]
                        )
                        nc.scalar.activation(
                            mt[c][:, q * 125 : (q + 1) * 125],
                            tp[:cw, :],
                            mybir.ActivationFunctionType.Copy,
                            scale=1.0 / APM,
                        )
                # h0T = relu(W_fc0^T @ mvT + b_fc0): 4 M-chunks x 3 K-chunks
                h0 = []
                for m in range(4):
                    hp = ps.tile([128, MK], F32, name="h0p", tag="h0p")
                    for c, (c0, cw) in enumerate(((0, 128), (128, 128), (256, 44))):
                        wt = sb.tile([cw, 128], BF16, name="w0t", tag="w0t",
                                     padded_shape=[128, None])
                        nc.sync.dma_start(
                            out=wt[:], in_=wfc0[c0 : c0 + cw, m * 128 : (m + 1) * 128]
                        )
                        nc.tensor.matmul(
                            hp[:], wt[:cw, :], mt[c][:cw, :], start=(c == 0), stop=(c == 2)
                        )
                    bt = sb.tile([128, 1], F32, name="b0t", tag="b0t")
                    nc.sync.dma_start(out=bt[:], in_=bfc0[m * 128 : (m + 1) * 128, :])
                    ht = sb.tile([128, MK], BF16, name=f"h0_{m}")
                    nc.scalar.activation(
                        ht[:], hp[:], mybir.ActivationFunctionType.Relu, bias=bt[:]
                    )
                    h0.append(ht)
                # h1T = relu(W_fc1^T @ h0T + b_fc1)
                h1 = []
                for m in range(4):
                    hp = ps.tile([128, MK], F32, name="h1p", tag="h1p")
                    for c in range(4):
                        wt = sb.tile([128, 128], BF16, name="w1t", tag="w1t")
                        nc.sync.dma_start(
                            out=wt[:],
                            in_=wfc1[c * 128 : (c + 1) * 128, m * 128 : (m + 1) * 128],
                        )
                        nc.tensor.matmul(
                            hp[:], wt[:], h1_src(h0, c), start=(c == 0), stop=(c == 3)
                        )
                    bt = sb.tile([128, 1], F32, name="b1t", tag="b1t")
                    nc.sync.dma_start(out=bt[:], in_=bfc1[m * 128 : (m + 1) * 128, :])
                    ht = sb.tile([128, MK], BF16, name=f"h1_{m}")
                    nc.scalar.activation(
                        ht[:], hp[:], mybir.ActivationFunctionType.Relu, bias=bt[:]
                    )
                    h1.append(ht)
                # out = W_last^T @ h1T + b_last
                op = ps.tile([1, MK], F32, name="op", tag="op")
                for c in range(4):
                    wt = sb.tile([128, 1], BF16, name="wlt", tag="wlt")
                    nc.sync.dma_start(out=wt[:], in_=wlast[c * 128 : (c + 1) * 128, :])
                    nc.tensor.matmul(
                        op[:], wt[:], h1[c][:], start=(c == 0), stop=(c == 3)
                    )
                blt = sb.tile([1, 1], F32, name="blt")
                nc.sync.dma_start(out=blt[:], in_=blast[:])
                outs = sb.tile([1, MK], F32, name="outs")
                nc.vector.tensor_tensor(
                    out=outs[:], in0=op[:], in1=blt[:].to_broadcast([1, MK]),
                    op=mybir.AluOpType.add,
                )
                nc.sync.dma_start(out=out_ext[:], in_=outs[:])

    nc.compile()
    return nc


def h1_src(h0, c):
    return h0[c][:]


def _prep_inputs(inputs):
    """Host-side sharding/preprocessing. Index-only work plus dtype casts."""
    f_atoms = np.asarray(inputs["f_atoms"], np.float32)
    f_bonds = np.asarray(inputs["f_bonds"], np.float32)
    a2b = np.asarray(inputs["a2b"], np.int64)
    b2a = np.asarray(inputs["b2a"], np.int64)
    b2revb = np.asarray(inputs["b2revb"], np.int64)

    # map global bond id -> Z row (core k slice is padded to BKP rows)
    def zrow(idx):
        return ((idx // BK) * BKP + (idx % BK)).astype(np.int32)

    nbr = a2b[b2a]  # [B, 6] bond ids feeding each bond's source atom
    it_idx_g = np.concatenate([zrow(nbr), zrow(b2revb)[:, None]], axis=1)  # [B,7]
    fin_idx_g = zrow(a2b)  # [A, 6]

    w = {}
    w["wi"] = np.asarray(inputs["W_i"], np.float32).astype(bf16)
    w["wh"] = np.asarray(inputs["W_h"], np.float32).astype(bf16)
    W_o = np.asarray(inputs["W_o"], np.float32)
    w["wo1"] = W_o[:ATOM_F].astype(bf16)
    w["wo2"] = W_o[ATOM_F:].astype(bf16)
    w["wfc0"] = np.asarray(inputs["W_fc0"], np.float32).astype(bf16)
    w["wfc1"] = np.asarray(inputs["W_fc1"], np.float32).astype(bf16)
    w["wlast"] = np.asarray(inputs["W_last"], np.float32).astype(bf16)
    w["bi_r"] = np.tile(np.asarray(inputs["b_i"], np.float32)[None, :], (128, 1))
    w["bh_r"] = np.tile(np.asarray(inputs["b_h"], np.float32)[None, :], (128, 1))
    w["bo_r"] = np.tile(np.asarray(inputs["b_o"], np.float32)[None, :], (128, 1))
    w["bfc0"] = np.asarray(inputs["b_fc0"], np.float32).reshape(RO, 1)
    w["bfc1"] = np.asarray(inputs["b_fc1"], np.float32).reshape(RO, 1)
    w["blast"] = np.asarray(inputs["b_last"], np.float32).reshape(1, 1)
    psel = np.zeros((125, 5), np.float32)
    psel[np.arange(125), np.arange(125) // APM] = 1.0
    w["psel_in"] = psel.astype(bf16)

    in_maps = []
    for k in range(NCORES):
        bs, be = k * BK, (k + 1) * BK
        as_, ae = k * AK, (k + 1) * AK
        fbt = np.zeros((BOND_F, BKP), np.float32)
        fbt[:, :BK] = f_bonds[bs:be].T
        fat = f_atoms[as_:ae].T.copy()
        iti = np.zeros((BKP, 7), np.int32)
        iti[:BK] = it_idx_g[bs:be]
        m = dict(w)
        m["fbT"] = fbt.astype(bf16).reshape(BOND_F, BT, 128)
        m["faT"] = fat.astype(bf16).reshape(ATOM_F, AT, 125)
        m["it_idx"] = iti.reshape(BT, 128, 7)
        m["fin_idx"] = fin_idx_g[as_:ae].astype(np.int32).reshape(AT, 125, 6)
        in_maps.append(m)
    return in_maps


def kernel(**inputs) -> np.ndarray:
    global _NC_CACHE
    if _NC_CACHE is None:
        _NC_CACHE = build()
    nc = _NC_CACHE
    in_maps = _prep_inputs(inputs)
    res = run_bass_kernel_spmd(nc, in_maps, core_ids=list(range(NCORES)))
    out = np.concatenate(
        [res.results[k]["out"].reshape(-1) for k in range(NCORES)], axis=0
    )
    return out.astype(np.float32)


# revision 7
# speedup vs baseline: 30.5499x; 30.5499x over previous
import os, sys
import numpy as np
import ml_dtypes

sys.path.insert(0, "/opt/trn_rl_repo")
sys.path.insert(0, "/opt/trn_rl_repo/concourse")

import concourse.bass as bass
import concourse.bacc as bacc
import concourse.tile as tile
import concourse.mybir as mybir
from concourse.bass_utils import run_bass_kernel_spmd
from concourse.masks import make_identity

BF16 = mybir.dt.bfloat16
F32 = mybir.dt.float32
I32 = mybir.dt.int32
bf16 = ml_dtypes.bfloat16

NCORES = 8
A, B, NB = 100000, 200000, 6
BOND_F, ATOM_F, H, RO = 147, 133, 300, 512
DEPTH = 6
APM = 25  # atoms per mol
BK = B // NCORES  # 25000 bonds/core
AK = A // NCORES  # 12500 atoms/core
MK = AK // APM  # 500 mols/core
BT = 196  # bond tiles of 128 (196*128 = 25088 >= 25000)
BKP = BT * 128  # padded bonds per core
AT = 100  # atom tiles of 125 (100*125 = 12500)
ZROWS = NCORES * BKP
UNROLL = 4

_NC_CACHE = None
SKIP_GATHERS = bool(int(os.environ.get("K_SKIP_GATHERS", "0")))
SKIP_AGS = bool(int(os.environ.get("K_SKIP_AGS", "0")))
SKIP_TAIL = bool(int(os.environ.get("K_SKIP_TAIL", "0")))


def _emit_bond_tail(nc, sb, ps, ident, wh_c, zloc_v, ti, msg_t):
    """Transpose msg tile and matmul against weight chunks; write Z row tile."""
    zp = ps.tile([128, H], F32, name="zp", tag="zp")
    for c, (c0, cw) in enumerate(((0, 128), (128, 128), (256, 44))):
        tp = ps.tile([128, 128], BF16, name="tp", tag="tp")
        nc.tensor.transpose(tp[:cw, :], msg_t[:, c0 : c0 + cw], ident[:])
        mT = sb.tile([128, 128], BF16, name="mT", tag="mT")
        nc.vector.tensor_copy(mT[:cw, :], tp[:cw, :])
        nc.tensor.matmul(
            zp[:], mT[:cw, :], wh_c[c], start=(c == 0), stop=(c == 2)
        )
    zs = sb.tile([128, H], BF16, name="zs", tag="zs")
    nc.vector.tensor_copy(zs[:], zp[:])
    nc.sync.dma_start(out=zloc_v[ti], in_=zs[:])


def build():
    nc = bacc.Bacc("TRN2", target_bir_lowering=False, debug=False, num_devices=NCORES)

    # ---------------- inputs ----------------
    fbT = nc.dram_tensor("fbT", [BOND_F, BT, 128], BF16, kind="ExternalInput")
    faT = nc.dram_tensor("faT", [ATOM_F, AT, 125], BF16, kind="ExternalInput")
    it_idx = nc.dram_tensor("it_idx", [BT, 128, 7], I32, kind="ExternalInput")
    fin_idx = nc.dram_tensor("fin_idx", [AT, 125, 6], I32, kind="ExternalInput")
    wi = nc.dram_tensor("wi", [BOND_F, H], BF16, kind="ExternalInput")
    wh = nc.dram_tensor("wh", [H, H], BF16, kind="ExternalInput")
    wo1 = nc.dram_tensor("wo1", [ATOM_F, H], BF16, kind="ExternalInput")
    wo2 = nc.dram_tensor("wo2", [H, H], BF16, kind="ExternalInput")
    wfc0 = nc.dram_tensor("wfc0", [H, RO], BF16, kind="ExternalInput")
    wfc1 = nc.dram_tensor("wfc1", [RO, RO], BF16, kind="ExternalInput")
    wlast = nc.dram_tensor("wlast", [RO, 1], BF16, kind="ExternalInput")
    bi_r = nc.dram_tensor("bi_r", [128, H], F32, kind="ExternalInput")
    bh_r = nc.dram_tensor("bh_r", [128, H], F32, kind="ExternalInput")
    bo_r = nc.dram_tensor("bo_r", [128, H], F32, kind="ExternalInput")
    bfc0 = nc.dram_tensor("bfc0", [RO, 1], F32, kind="ExternalInput")
    bfc1 = nc.dram_tensor("bfc1", [RO, 1], F32, kind="ExternalInput")
    blast = nc.dram_tensor("blast", [1, 1], F32, kind="ExternalInput")
    psel_in = nc.dram_tensor("psel_in", [125, 5], BF16, kind="ExternalInput")
    out_ext = nc.dram_tensor("out", [1, MK], F32, kind="ExternalOutput")

    with tile.TileContext(nc) as tc:
        with (
            tc.tile_pool(name="const", bufs=1) as cst,
            tc.tile_pool(name="dram", bufs=1, space="DRAM") as dram,
        ):
            # big DRAM buffers
            zbufs = [
                dram.tile([ZROWS, H], BF16, addr_space="Shared", name=f"zfull{t}")
                for t in range(DEPTH)
            ]
            zloc = dram.tile([BKP, H], BF16, name="zloc")
            inpb_d = dram.tile([BT, 128, H], BF16, name="inpb_d")
            mv_d = dram.tile([AT, 5, H], F32, name="mv_d")
            zloc_v = zloc[:].rearrange("(t p) h -> t p h", p=128)

            # resident constants
            ident = cst.tile([128, 128], BF16, name="ident")
            make_identity(nc, ident[:])
            wi_c0 = cst.tile([128, H], BF16, name="wi_c0")
            wi_c1 = cst.tile([19, H], BF16, name="wi_c1", padded_shape=[128, None])
            nc.sync.dma_start(out=wi_c0[:], in_=wi[0:128, :])
            nc.sync.dma_start(out=wi_c1[:], in_=wi[128:BOND_F, :])
            wh_c, wo2_c = [], []
            for c, (c0, cw) in enumerate(((0, 128), (128, 128), (256, 44))):
                t1 = cst.tile([cw, H], BF16, name=f"wh_c{c}", padded_shape=[128, None])
                nc.sync.dma_start(out=t1[:], in_=wh[c0 : c0 + cw, :])
                wh_c.append(t1[:])
                t2 = cst.tile([cw, H], BF16, name=f"wo2_c{c}", padded_shape=[128, None])
                nc.sync.dma_start(out=t2[:], in_=wo2[c0 : c0 + cw, :])
                wo2_c.append(t2[:])
            wo1_c0 = cst.tile([128, H], BF16, name="wo1_c0")
            wo1_c1 = cst.tile([5, H], BF16, name="wo1_c1", padded_shape=[128, None])
            nc.sync.dma_start(out=wo1_c0[:], in_=wo1[0:128, :])
            nc.sync.dma_start(out=wo1_c1[:], in_=wo1[128:ATOM_F, :])
            bi_t = cst.tile([128, H], F32, name="bi_t")
            bh_t = cst.tile([128, H], F32, name="bh_t")
            bo_t = cst.tile([128, H], F32, name="bo_t")
            nc.sync.dma_start(out=bi_t[:], in_=bi_r[:])
            nc.sync.dma_start(out=bh_t[:], in_=bh_r[:])
            nc.sync.dma_start(out=bo_t[:], in_=bo_r[:])
            psel = cst.tile([125, 5], BF16, name="psel")
            nc.sync.dma_start(out=psel[:], in_=psel_in[:])

            rg = [list(range(NCORES))]

            # ---------------- stage 0: inp/msg0/Z0 ----------------
            with (
                tc.tile_pool(name="s0sb", bufs=3) as sb,
                tc.tile_pool(name="s0ps", bufs=2, space="PSUM") as ps,
            ):
                with tc.For_i(0, BT, UNROLL) as i0:
                    for u in range(UNROLL):
                        ti = i0 + u
                        fb0 = sb.tile([128, 128], BF16, name="fb0", tag="fb0")
                        fb1 = sb.tile(
                            [19, 128], BF16, name="fb1", tag="fb1",
                            padded_shape=[128, None],
                        )
                        nc.sync.dma_start(out=fb0[:], in_=fbT[0:128, ti, :])
                        nc.sync.dma_start(out=fb1[:], in_=fbT[128:BOND_F, ti, :])
                        ip = ps.tile([128, H], F32, name="ip", tag="ip")
                        nc.tensor.matmul(ip[:], fb0[:], wi_c0[:], start=True, stop=False)
                        nc.tensor.matmul(
                            ip[:], fb1[:], wi_c1[:19, :], start=False, stop=True
                        )
                        t0 = sb.tile([128, H], F32, name="t0", tag="t0")
                        nc.vector.tensor_tensor(
                            out=t0[:], in0=ip[:], in1=bi_t[:], op=mybir.AluOpType.add
                        )
                        msg_t = sb.tile([128, H], BF16, name="msg_t", tag="msg_t")
                        nc.scalar.activation(
                            msg_t[:], t0[:], mybir.ActivationFunctionType.Relu
                        )
                        inb = sb.tile([128, H], BF16, name="inb", tag="inb")
                        nc.vector.tensor_tensor(
                            out=inb[:], in0=t0[:], in1=bh_t[:], op=mybir.AluOpType.add
                        )
                        nc.sync.dma_start(out=inpb_d[ti], in_=inb[:])
                        _emit_bond_tail(nc, sb, ps, ident, wh_c, zloc_v, ti, msg_t)
            if SKIP_AGS:
                nc.sync.dma_start(out=zbufs[0][0:BKP, :], in_=zloc[:])
            else:
                nc.gpsimd.collective_compute(
                    "AllGather", mybir.AluOpType.bypass, replica_groups=rg,
                    ins=[zloc[:].opt()], outs=[zbufs[0][:].opt()],
                )

            # ---------------- iterations 1..5 ----------------
            srcs = [zbufs[t] for t in range(DEPTH - 1)]
            dsts = [zbufs[t + 1] for t in range(DEPTH - 1)]
            for it in range(DEPTH - 1):
                last = it == DEPTH - 2
                w_chunks = wo2_c if last else wh_c
                src = srcs[it]
                with (
                    tc.tile_pool(name=f"i{it}sb", bufs=3) as sb,
                    tc.tile_pool(name=f"i{it}ps", bufs=2, space="PSUM") as ps,
                ):
                    with tc.For_i(0, BT, UNROLL) as iv:
                        for u in range(UNROLL):
                            ti = iv + u
                            ix = sb.tile([128, 7], I32, name="ix", tag="ix")
                            nc.sync.dma_start(out=ix[:], in_=it_idx[ti])
                            g = sb.tile([128, 7, H], BF16, name="g", tag="g")
                            for j in range(7 * (0 if SKIP_GATHERS else 1)):
                                nc.gpsimd.indirect_dma_start(
                                    out=g[:, j, :],
                                    out_offset=None,
                                    in_=src[:],
                                    in_offset=bass.IndirectOffsetOnAxis(
                                        ap=ix[:, j : j + 1], axis=0
                                    ),
                                )
                            if SKIP_GATHERS:
                                nc.gpsimd.memset(g[:, 0, :], 0.01)
                            acc = sb.tile([128, H], F32, name="acc", tag="acc")
                            nc.vector.reduce_sum(
                                acc[:],
                                g[:, 0:6, :].rearrange("p j h -> p h j"),
                                axis=mybir.AxisListType.X,
                            )
                            nc.vector.tensor_tensor(
                                out=acc[:], in0=acc[:], in1=g[:, 6, :],
                                op=mybir.AluOpType.subtract,
                            )
                            inb = sb.tile([128, H], BF16, name="inb", tag="inb")
                            nc.sync.dma_start(out=inb[:], in_=inpb_d[ti])
                            nc.vector.tensor_tensor(
                                out=acc[:], in0=acc[:], in1=inb[:],
                                op=mybir.AluOpType.add,
                            )
                            msg_t = sb.tile([128, H], BF16, name="msg_t", tag="msg_t")
                            nc.scalar.activation(
                                msg_t[:], acc[:], mybir.ActivationFunctionType.Relu
                            )
                            _emit_bond_tail(nc, sb, ps, ident, w_chunks, zloc_v, ti, msg_t)
                if SKIP_AGS:
                    nc.sync.dma_start(out=dsts[it][0:BKP, :], in_=zloc[:])
                else:
                    nc.gpsimd.collective_compute(
                        "AllGather", mybir.AluOpType.bypass, replica_groups=rg,
                        ins=[zloc[:].opt()], outs=[dsts[it][:].opt()],
                    )

            # ---------------- final atom stage ----------------
            zo = dsts[DEPTH - 2]  # gathered ZO table
            with (
                tc.tile_pool(name="fsb", bufs=3) as sb,
                tc.tile_pool(name="fps", bufs=2, space="PSUM") as ps,
            ):
                with tc.For_i(0, AT, UNROLL) as fv:
                    for u in range(UNROLL):
                        ti = fv + u
                        ix = sb.tile([125, 6], I32, name="fix", tag="fix")
                        nc.sync.dma_start(out=ix[:], in_=fin_idx[ti])
                        g = sb.tile([125, 6, H], BF16, name="fg", tag="fg")
                        if SKIP_GATHERS:
                            nc.gpsimd.memset(g[:, 0, :], 0.01)
                        for j in range(6 * (0 if SKIP_GATHERS else 1)):
                            nc.gpsimd.indirect_dma_start(
                                out=g[:, j, :],
                                out_offset=None,
                                in_=zo[:],
                                in_offset=bass.IndirectOffsetOnAxis(
                                    ap=ix[:, j : j + 1], axis=0
                                ),
                            )
                        acc = sb.tile([125, H], F32, name="facc", tag="facc")
                        nc.vector.reduce_sum(
                            acc[:],
                            g[:].rearrange("p j h -> p h j"),
                            axis=mybir.AxisListType.X,
                        )
                        fa0 = sb.tile([128, 125], BF16, name="fa0", tag="fa0")
                        fa1 = sb.tile(
                            [5, 125], BF16, name="fa1", tag="fa1",
                            padded_shape=[128, None],
                        )
                        nc.sync.dma_start(out=fa0[:], in_=faT[0:128, ti, :])
                        nc.sync.dma_start(out=fa1[:], in_=faT[128:ATOM_F, ti, :])
                        ap_ = ps.tile([125, H], F32, name="ap_", tag="ap_")
                        nc.tensor.matmul(ap_[:], fa0[:, :], wo1_c0[:], start=True, stop=False)
                        nc.tensor.matmul(ap_[:], fa1[:5, :], wo1_c1[:5, :], start=False, stop=True)
                        nc.vector.tensor_tensor(
                            out=acc[:], in0=acc[:], in1=ap_[:], op=mybir.AluOpType.add
                        )
                        nc.vector.tensor_tensor(
                            out=acc[:], in0=acc[:], in1=bo_t[:125, :], op=mybir.AluOpType.add
                        )
                        ah = sb.tile([125, H], BF16, name="ah", tag="ah")
                        nc.scalar.activation(
                            ah[:], acc[:], mybir.ActivationFunctionType.Relu
                        )
                        mvp = ps.tile([5, H], F32, name="mvp", tag="mvp")
                        nc.tensor.matmul(mvp[:], psel[:], ah[:], start=True, stop=True)
                        mvs = sb.tile([5, H], F32, name="mvs", tag="mvs")
                        nc.vector.tensor_copy(mvs[:], mvp[:])
                        nc.sync.dma_start(out=mv_d[ti], in_=mvs[:])

            # ---------------- readout (static) ----------------
            with (
                tc.tile_pool(name="rsb", bufs=1) as sb,
                tc.tile_pool(name="rps", bufs=1, space="PSUM") as ps,
            ):
                # build mvT [300, 500] as 3 sbuf tiles [cw, 500], scaled by 1/APM
                mt = []
                for c, (c0, cw) in enumerate(((0, 128), (128, 128), (256, 44))):
                    t = sb.tile([cw, MK], BF16, name=f"mt{c}", padded_shape=[128, None])
                    mt.append(t)
                for q in range(4):
                    mvq = sb.tile([125, H], F32, name=f"mvq{q}")
                    nc.sync.dma_start(
                        out=mvq[:],
                        in_=mv_d[:].rearrange("t f h -> (t f) h")[
                            q * 125 : (q + 1) * 125, :
                        ],
                    )
                    mvqb = sb.tile([125, H], BF16, name=f"mvqb{q}")
                    nc.vector.tensor_copy(mvqb[:], mvq[:])
                    for c, (c0, cw) in enumerate(((0, 128), (128, 128), (256, 44))):
                        tp = ps.tile([128, 125], BF16, name="rtp", tag="rtp")
                        nc.tensor.transpose(
                            tp[:cw, :], mvqb[:, c0 : c0 + cw], ident[:125, :125]
                        )
                        nc.scalar.activation(
                            mt[c][:, q * 125 : (q + 1) * 125],
                            tp[:cw, :],
                            mybir.ActivationFunctionType.Copy,
                            scale=1.0 / APM,
                        )
                # h0T = relu(W_fc0^T @ mvT + b_fc0): 4 M-chunks x 3 K-chunks
                h0 = []
                for m in range(4):
                    hp = ps.tile([128, MK], F32, name="h0p", tag="h0p")
                    for c, (c0, cw) in enumerate(((0, 128), (128, 128), (256, 44))):
                        wt = sb.tile([cw, 128], BF16, name="w0t", tag="w0t",
                                     padded_shape=[128, None])
                        nc.sync.dma_start(
                            out=wt[:], in_=wfc0[c0 : c0 + cw, m * 128 : (m + 1) * 128]
                        )
                        nc.tensor.matmul(
                            hp[:], wt[:cw, :], mt[c][:cw, :], start=(c == 0), stop=(c == 2)
                        )
                    bt = sb.tile([128, 1], F32, name="b0t", tag="b0t")
                    nc.sync.dma_start(out=bt[:], in_=bfc0[m * 128 : (m + 1) * 128, :])
                    ht = sb.tile([128, MK], BF16, name=f"h0_{m}")
                    nc.scalar.activation(
                        ht[:], hp[:], mybir.ActivationFunctionType.Relu, bias=bt[:]
                    )
                    h0.append(ht)
                # h1T = relu(W_fc1^T @ h0T + b_fc1)
                h1 = []
                for m in range(4):
                    hp = ps.tile([128, MK], F32, name="h1p", tag="h1p")
                    for c in range(4):
                        wt = sb.tile([128, 128], BF16, name="w1t", tag="w1t")
                        nc.sync.dma_start(
                            out=wt[:],
                            in_=wfc1[c * 128 : (c + 1) * 128, m * 128 : (m + 1) * 128],
                        )
                        nc.tensor.matmul(
                            hp[:], wt[:], h1_src(h0, c), start=(c == 0), stop=(c == 3)
                        )
                    bt = sb.tile([128, 1], F32, name="b1t", tag="b1t")
                    nc.sync.dma_start(out=bt[:], in_=bfc1[m * 128 : (m + 1) * 128, :])
                    ht = sb.tile([128, MK], BF16, name=f"h1_{m}")
                    nc.scalar.activation(
                        ht[:], hp[:], mybir.ActivationFunctionType.Relu, bias=bt[:]
                    )
                    h1.append(ht)
                # out = W_last^T @ h1T + b_last
                op = ps.tile([1, MK], F32, name="op", tag="op")
                for c in range(4):
                    wt = sb.tile([128, 1], BF16, name="wlt", tag="wlt")
                    nc.sync.dma_start(out=wt[:], in_=wlast[c * 128 : (c + 1) * 128, :])
                    nc.tensor.matmul(
                        op[:], wt[:], h1[c][:], start=(c == 0), stop=(c == 3)
                    )
                blt = sb.tile([1, 1], F32, name="blt")
                nc.sync.dma_start(out=blt[:], in_=blast[:])
                outs = sb.tile([1, MK], F32, name="outs")
                nc.vector.tensor_tensor(
                    out=outs[:], in0=op[:], in1=blt[:].to_broadcast([1, MK]),
                    op=mybir.AluOpType.add,
                )
                nc.sync.dma_start(out=out_ext[:], in_=outs[:])

    nc.compile()
    return nc


def h1_src(h0, c):
    return h0[c][:]


def _prep_inputs(inputs):
    """Host-side sharding/preprocessing. Index-only work plus dtype casts."""
    f_atoms = np.asarray(inputs["f_atoms"], np.float32)
    f_bonds = np.asarray(inputs["f_bonds"], np.float32)
    a2b = np.asarray(inputs["a2b"], np.int64)
    b2a = np.asarray(inputs["b2a"], np.int64)
    b2revb = np.asarray(inputs["b2revb"], np.int64)

    # map global bond id -> Z row (core k slice is padded to BKP rows)
    def zrow(idx):
        return ((idx // BK) * BKP + (idx % BK)).astype(np.int32)

    nbr = a2b[b2a]  # [B, 6] bond ids feeding each bond's source atom
    it_idx_g = np.concatenate([zrow(nbr), zrow(b2revb)[:, None]], axis=1)  # [B,7]
    fin_idx_g = zrow(a2b)  # [A, 6]

    w = {}
    w["wi"] = np.asarray(inputs["W_i"], np.float32).astype(bf16)
    w["wh"] = np.asarray(inputs["W_h"], np.float32).astype(bf16)
    W_o = np.asarray(inputs["W_o"], np.float32)
    w["wo1"] = W_o[:ATOM_F].astype(bf16)
    w["wo2"] = W_o[ATOM_F:].astype(bf16)
    w["wfc0"] = np.asarray(inputs["W_fc0"], np.float32).astype(bf16)
    w["wfc1"] = np.asarray(inputs["W_fc1"], np.float32).astype(bf16)
    w["wlast"] = np.asarray(inputs["W_last"], np.float32).astype(bf16)
    w["bi_r"] = np.tile(np.asarray(inputs["b_i"], np.float32)[None, :], (128, 1))
    w["bh_r"] = np.tile(np.asarray(inputs["b_h"], np.float32)[None, :], (128, 1))
    w["bo_r"] = np.tile(np.asarray(inputs["b_o"], np.float32)[None, :], (128, 1))
    w["bfc0"] = np.asarray(inputs["b_fc0"], np.float32).reshape(RO, 1)
    w["bfc1"] = np.asarray(inputs["b_fc1"], np.float32).reshape(RO, 1)
    w["blast"] = np.asarray(inputs["b_last"], np.float32).reshape(1, 1)
    psel = np.zeros((125, 5), np.float32)
    psel[np.arange(125), np.arange(125) // APM] = 1.0
    w["psel_in"] = psel.astype(bf16)

    in_maps = []
    for k in range(NCORES):
        bs, be = k * BK, (k + 1) * BK
        as_, ae = k * AK, (k + 1) * AK
        fbt = np.zeros((BOND_F, BKP), np.float32)
        fbt[:, :BK] = f_bonds[bs:be].T
        fat = f_atoms[as_:ae].T.copy()
        iti = np.zeros((BKP, 7), np.int32)
        iti[:BK] = it_idx_g[bs:be]
        m = dict(w)
        m["fbT"] = fbt.astype(bf16).reshape(BOND_F, BT, 128)
        m["faT"] = fat.astype(bf16).reshape(ATOM_F, AT, 125)
        m["it_idx"] = iti.reshape(BT, 128, 7)
        m["fin_idx"] = fin_idx_g[as_:ae].astype(np.int32).reshape(AT, 125, 6)
        in_maps.append(m)
    return in_maps


def kernel(**inputs) -> np.ndarray:
    global _NC_CACHE
    if _NC_CACHE is None:
        _NC_CACHE = build()
    nc = _NC_CACHE
    in_maps = _prep_inputs(inputs)
    res = run_bass_kernel_spmd(nc, in_maps, core_ids=list(range(NCORES)))
    out = np.concatenate(
        [res.results[k]["out"].reshape(-1) for k in range(NCORES)], axis=0
    )
    return out.astype(np.float32)


# revision 8
# speedup vs baseline: 30.7033x; 1.0050x over previous
import os, sys
import numpy as np
import ml_dtypes

sys.path.insert(0, "/opt/trn_rl_repo")
sys.path.insert(0, "/opt/trn_rl_repo/concourse")

import concourse.bass as bass
import concourse.bacc as bacc
import concourse.tile as tile
import concourse.mybir as mybir
from concourse.bass_utils import run_bass_kernel_spmd
from concourse.masks import make_identity

BF16 = mybir.dt.bfloat16
F32 = mybir.dt.float32
I32 = mybir.dt.int32
bf16 = ml_dtypes.bfloat16

NCORES = 8
A, B, NB = 100000, 200000, 6
BOND_F, ATOM_F, H, RO = 147, 133, 300, 512
DEPTH = 6
APM = 25  # atoms per mol
BK = B // NCORES  # 25000 bonds/core
AK = A // NCORES  # 12500 atoms/core
MK = AK // APM  # 500 mols/core
BT = 196  # bond tiles of 128 (196*128 = 25088 >= 25000)
BKP = BT * 128  # padded bonds per core
AT = 100  # atom tiles of 125 (100*125 = 12500)
ZROWS = NCORES * BKP
UNROLL = 4

_NC_CACHE = None
SKIP_GATHERS = bool(int(os.environ.get("K_SKIP_GATHERS", "0")))
SKIP_AGS = bool(int(os.environ.get("K_SKIP_AGS", "0")))
SKIP_TAIL = bool(int(os.environ.get("K_SKIP_TAIL", "0")))


def _emit_bond_tail(nc, sb, ps, ident, wh_c, zloc_v, ti, msg_t):
    """Transpose msg tile and matmul against weight chunks; write Z row tile."""
    zp = ps.tile([128, H], F32, name="zp", tag="zp")
    for c, (c0, cw) in enumerate(((0, 128), (128, 128), (256, 44))):
        tp = ps.tile([128, 128], BF16, name="tp", tag="tp")
        nc.tensor.transpose(tp[:cw, :], msg_t[:, c0 : c0 + cw], ident[:])
        mT = sb.tile([128, 128], BF16, name="mT", tag="mT")
        nc.vector.tensor_copy(mT[:cw, :], tp[:cw, :])
        nc.tensor.matmul(
            zp[:], mT[:cw, :], wh_c[c], start=(c == 0), stop=(c == 2)
        )
    zs = sb.tile([128, H], BF16, name="zs", tag="zs")
    nc.vector.tensor_copy(zs[:], zp[:])
    nc.sync.dma_start(out=zloc_v[ti], in_=zs[:])


def build():
    nc = bacc.Bacc("TRN2", target_bir_lowering=False, debug=False, num_devices=NCORES)

    # ---------------- inputs ----------------
    fbT = nc.dram_tensor("fbT", [BOND_F, BT, 128], BF16, kind="ExternalInput")
    faT = nc.dram_tensor("faT", [ATOM_F, AT, 125], BF16, kind="ExternalInput")
    it_idx = nc.dram_tensor("it_idx", [BT, 128, 7], I32, kind="ExternalInput")
    fin_idx = nc.dram_tensor("fin_idx", [AT, 125, 6], I32, kind="ExternalInput")
    wi = nc.dram_tensor("wi", [BOND_F, H], BF16, kind="ExternalInput")
    wh = nc.dram_tensor("wh", [H, H], BF16, kind="ExternalInput")
    wo1 = nc.dram_tensor("wo1", [ATOM_F, H], BF16, kind="ExternalInput")
    wo2 = nc.dram_tensor("wo2", [H, H], BF16, kind="ExternalInput")
    wfc0 = nc.dram_tensor("wfc0", [H, RO], BF16, kind="ExternalInput")
    wfc1 = nc.dram_tensor("wfc1", [RO, RO], BF16, kind="ExternalInput")
    wlast = nc.dram_tensor("wlast", [RO, 1], BF16, kind="ExternalInput")
    bi_r = nc.dram_tensor("bi_r", [128, H], F32, kind="ExternalInput")
    bh_r = nc.dram_tensor("bh_r", [128, H], F32, kind="ExternalInput")
    bo_r = nc.dram_tensor("bo_r", [128, H], F32, kind="ExternalInput")
    bfc0 = nc.dram_tensor("bfc0", [RO, 1], F32, kind="ExternalInput")
    bfc1 = nc.dram_tensor("bfc1", [RO, 1], F32, kind="ExternalInput")
    blast = nc.dram_tensor("blast", [1, 1], F32, kind="ExternalInput")
    psel_in = nc.dram_tensor("psel_in", [125, 5], BF16, kind="ExternalInput")
    out_ext = nc.dram_tensor("out", [1, MK], F32, kind="ExternalOutput")

    with tile.TileContext(nc) as tc:
        with (
            tc.tile_pool(name="const", bufs=1) as cst,
            tc.tile_pool(name="dram", bufs=1, space="DRAM") as dram,
        ):
            # big DRAM buffers
            zbufs = [
                dram.tile([ZROWS, H], BF16, addr_space="Shared", name=f"zfull{t}")
                for t in range(DEPTH)
            ]
            zloc = dram.tile([BKP, H], BF16, name="zloc")
            inpb_d = dram.tile([BT, 128, H], BF16, name="inpb_d")
            mv_d = dram.tile([AT, 5, H], F32, name="mv_d")
            zloc_v = zloc[:].rearrange("(t p) h -> t p h", p=128)

            # resident constants
            ident = cst.tile([128, 128], BF16, name="ident")
            make_identity(nc, ident[:])
            wi_c0 = cst.tile([128, H], BF16, name="wi_c0")
            wi_c1 = cst.tile([19, H], BF16, name="wi_c1", padded_shape=[128, None])
            nc.sync.dma_start(out=wi_c0[:], in_=wi[0:128, :])
            nc.sync.dma_start(out=wi_c1[:], in_=wi[128:BOND_F, :])
            wh_c, wo2_c = [], []
            for c, (c0, cw) in enumerate(((0, 128), (128, 128), (256, 44))):
                t1 = cst.tile([cw, H], BF16, name=f"wh_c{c}", padded_shape=[128, None])
                nc.sync.dma_start(out=t1[:], in_=wh[c0 : c0 + cw, :])
                wh_c.append(t1[:])
                t2 = cst.tile([cw, H], BF16, name=f"wo2_c{c}", padded_shape=[128, None])
                nc.sync.dma_start(out=t2[:], in_=wo2[c0 : c0 + cw, :])
                wo2_c.append(t2[:])
            wo1_c0 = cst.tile([128, H], BF16, name="wo1_c0")
            wo1_c1 = cst.tile([5, H], BF16, name="wo1_c1", padded_shape=[128, None])
            nc.sync.dma_start(out=wo1_c0[:], in_=wo1[0:128, :])
            nc.sync.dma_start(out=wo1_c1[:], in_=wo1[128:ATOM_F, :])
            bi_t = cst.tile([128, H], F32, name="bi_t")
            bh_t = cst.tile([128, H], F32, name="bh_t")
            bo_t = cst.tile([128, H], F32, name="bo_t")
            nc.sync.dma_start(out=bi_t[:], in_=bi_r[:])
            nc.sync.dma_start(out=bh_t[:], in_=bh_r[:])
            nc.sync.dma_start(out=bo_t[:], in_=bo_r[:])
            psel = cst.tile([125, 5], BF16, name="psel")
            nc.sync.dma_start(out=psel[:], in_=psel_in[:])

            rg = [list(range(NCORES))]

            # ---------------- stage 0: inp/msg0/Z0 ----------------
            with (
                tc.tile_pool(name="s0sb", bufs=4) as sb,
                tc.tile_pool(name="s0ps", bufs=2, space="PSUM") as ps,
            ):
                with tc.For_i(0, BT, UNROLL) as i0:
                    for u in range(UNROLL):
                        ti = i0 + u
                        fb0 = sb.tile([128, 128], BF16, name="fb0", tag="fb0")
                        fb1 = sb.tile(
                            [19, 128], BF16, name="fb1", tag="fb1",
                            padded_shape=[128, None],
                        )
                        nc.sync.dma_start(out=fb0[:], in_=fbT[0:128, ti, :])
                        nc.sync.dma_start(out=fb1[:], in_=fbT[128:BOND_F, ti, :])
                        ip = ps.tile([128, H], F32, name="ip", tag="ip")
                        nc.tensor.matmul(ip[:], fb0[:], wi_c0[:], start=True, stop=False)
                        nc.tensor.matmul(
                            ip[:], fb1[:], wi_c1[:19, :], start=False, stop=True
                        )
                        t0 = sb.tile([128, H], F32, name="t0", tag="t0")
                        nc.vector.tensor_tensor(
                            out=t0[:], in0=ip[:], in1=bi_t[:], op=mybir.AluOpType.add
                        )
                        msg_t = sb.tile([128, H], BF16, name="msg_t", tag="msg_t")
                        nc.scalar.activation(
                            msg_t[:], t0[:], mybir.ActivationFunctionType.Relu
                        )
                        inb = sb.tile([128, H], BF16, name="inb", tag="inb")
                        nc.vector.tensor_tensor(
                            out=inb[:], in0=t0[:], in1=bh_t[:], op=mybir.AluOpType.add
                        )
                        nc.sync.dma_start(out=inpb_d[ti], in_=inb[:])
                        _emit_bond_tail(nc, sb, ps, ident, wh_c, zloc_v, ti, msg_t)
            if SKIP_AGS:
                nc.sync.dma_start(out=zbufs[0][0:BKP, :], in_=zloc[:])
            else:
                nc.gpsimd.collective_compute(
                    "AllGather", mybir.AluOpType.bypass, replica_groups=rg,
                    ins=[zloc[:].opt()], outs=[zbufs[0][:].opt()],
                )

            # ---------------- iterations 1..5 ----------------
            srcs = [zbufs[t] for t in range(DEPTH - 1)]
            dsts = [zbufs[t + 1] for t in range(DEPTH - 1)]
            for it in range(DEPTH - 1):
                last = it == DEPTH - 2
                w_chunks = wo2_c if last else wh_c
                src = srcs[it]
                with (
                    tc.tile_pool(name=f"i{it}sb", bufs=4) as sb,
                    tc.tile_pool(name=f"i{it}ps", bufs=2, space="PSUM") as ps,
                ):
                    with tc.For_i(0, BT, UNROLL) as iv:
                        for u in range(UNROLL):
                            ti = iv + u
                            ix = sb.tile([128, 7], I32, name="ix", tag="ix")
                            nc.sync.dma_start(out=ix[:], in_=it_idx[ti])
                            g = sb.tile([128, 7, H], BF16, name="g", tag="g")
                            for j in range(7 * (0 if SKIP_GATHERS else 1)):
                                nc.gpsimd.indirect_dma_start(
                                    out=g[:, j, :],
                                    out_offset=None,
                                    in_=src[:],
                                    in_offset=bass.IndirectOffsetOnAxis(
                                        ap=ix[:, j : j + 1], axis=0
                                    ),
                                )
                            if SKIP_GATHERS:
                                nc.gpsimd.memset(g[:, 0, :], 0.01)
                            acc = sb.tile([128, H], F32, name="acc", tag="acc")
                            nc.vector.reduce_sum(
                                acc[:],
                                g[:, 0:6, :].rearrange("p j h -> p h j"),
                                axis=mybir.AxisListType.X,
                            )
                            nc.vector.tensor_tensor(
                                out=acc[:], in0=acc[:], in1=g[:, 6, :],
                                op=mybir.AluOpType.subtract,
                            )
                            inb = sb.tile([128, H], BF16, name="inb", tag="inb")
                            nc.sync.dma_start(out=inb[:], in_=inpb_d[ti])
                            nc.vector.tensor_tensor(
                                out=acc[:], in0=acc[:], in1=inb[:],
                                op=mybir.AluOpType.add,
                            )
                            msg_t = sb.tile([128, H], BF16, name="msg_t", tag="msg_t")
                            nc.scalar.activation(
                                msg_t[:], acc[:], mybir.ActivationFunctionType.Relu
                            )
                            _emit_bond_tail(nc, sb, ps, ident, w_chunks, zloc_v, ti, msg_t)
                if SKIP_AGS:
                    nc.sync.dma_start(out=dsts[it][0:BKP, :], in_=zloc[:])
                else:
                    nc.gpsimd.collective_compute(
                        "AllGather", mybir.AluOpType.bypass, replica_groups=rg,
                        ins=[zloc[:].opt()], outs=[dsts[it][:].opt()],
                    )

            # ---------------- final atom stage ----------------
            zo = dsts[DEPTH - 2]  # gathered ZO table
            with (
                tc.tile_pool(name="fsb", bufs=4) as sb,
                tc.tile_pool(name="fps", bufs=2, space="PSUM") as ps,
            ):
                with tc.For_i(0, AT, UNROLL) as fv:
                    for u in range(UNROLL):
                        ti = fv + u
                        ix = sb.tile([125, 6], I32, name="fix", tag="fix")
                        nc.sync.dma_start(out=ix[:], in_=fin_idx[ti])
                        g = sb.tile([125, 6, H], BF16, name="fg", tag="fg")
                        if SKIP_GATHERS:
                            nc.gpsimd.memset(g[:, 0, :], 0.01)
                        for j in range(6 * (0 if SKIP_GATHERS else 1)):
                            nc.gpsimd.indirect_dma_start(
                                out=g[:, j, :],
                                out_offset=None,
                                in_=zo[:],
                                in_offset=bass.IndirectOffsetOnAxis(
                                    ap=ix[:, j : j + 1], axis=0
                                ),
                            )
                        acc = sb.tile([125, H], F32, name="facc", tag="facc")
                        nc.vector.reduce_sum(
                            acc[:],
                            g[:].rearrange("p j h -> p h j"),
                            axis=mybir.AxisListType.X,
                        )
                        fa0 = sb.tile([128, 125], BF16, name="fa0", tag="fa0")
                        fa1 = sb.tile(
                            [5, 125], BF16, name="fa1", tag="fa1",
                            padded_shape=[128, None],
                        )
                        nc.sync.dma_start(out=fa0[:], in_=faT[0:128, ti, :])
                        nc.sync.dma_start(out=fa1[:], in_=faT[128:ATOM_F, ti, :])
                        ap_ = ps.tile([125, H], F32, name="ap_", tag="ap_")
                        nc.tensor.matmul(ap_[:], fa0[:, :], wo1_c0[:], start=True, stop=False)
                        nc.tensor.matmul(ap_[:], fa1[:5, :], wo1_c1[:5, :], start=False, stop=True)
                        nc.vector.tensor_tensor(
                            out=acc[:], in0=acc[:], in1=ap_[:], op=mybir.AluOpType.add
                        )
                        nc.vector.tensor_tensor(
                            out=acc[:], in0=acc[:], in1=bo_t[:125, :], op=mybir.AluOpType.add
                        )
                        ah = sb.tile([125, H], BF16, name="ah", tag="ah")
                        nc.scalar.activation(
                            ah[:], acc[:], mybir.ActivationFunctionType.Relu
                        )
                        mvp = ps.tile([5, H], F32, name="mvp", tag="mvp")
                        nc.tensor.matmul(mvp[:], psel[:], ah[:], start=True, stop=True)
                        mvs = sb.tile([5, H], F32, name="mvs", tag="mvs")
                        nc.vector.tensor_copy(mvs[:], mvp[:])
                        nc.sync.dma_start(out=mv_d[ti], in_=mvs[:])

            # ---------------- readout (static) ----------------
            with (
                tc.tile_pool(name="rsb", bufs=1) as sb,
                tc.tile_pool(name="rps", bufs=1, space="PSUM") as ps,
            ):
                # build mvT [300, 500] as 3 sbuf tiles [cw, 500], scaled by 1/APM
                mt = []
                for c, (c0, cw) in enumerate(((0, 128), (128, 128), (256, 44))):
                    t = sb.tile([cw, MK], BF16, name=f"mt{c}", padded_shape=[128, None])
                    mt.append(t)
                for q in range(4):
                    mvq = sb.tile([125, H], F32, name=f"mvq{q}")
                    nc.sync.dma_start(
                        out=mvq[:],
                        in_=mv_d[:].rearrange("t f h -> (t f) h")[
                            q * 125 : (q + 1) * 125, :
                        ],
                    )
                    mvqb = sb.tile([125, H], BF16, name=f"mvqb{q}")
                    nc.vector.tensor_copy(mvqb[:], mvq[:])
                    for c, (c0, cw) in enumerate(((0, 128), (128, 128), (256, 44))):
                        tp = ps.tile([128, 125], BF16, name="rtp", tag="rtp")
                        nc.tensor.transpose(
                            tp[:cw, :], mvqb[:, c0 : c0 + cw], ident[:125, :125]
                        )
                        nc.scalar.activation(
                            mt[c][:, q * 125 : (q + 1) * 125],
                            tp[:cw, :],
                            mybir.ActivationFunctionType.Copy,
                            scale=1.0 / APM,
                        )
                # h0T = relu(W_fc0^T @ mvT + b_fc0): 4 M-chunks x 3 K-chunks
                h0 = []
                for m in range(4):
                    hp = ps.tile([128, MK], F32, name="h0p", tag="h0p")
                    for c, (c0, cw) in enumerate(((0, 128), (128, 128), (256, 44))):
                        wt = sb.tile([cw, 128], BF16, name="w0t", tag="w0t",
                                     padded_shape=[128, None])
                        nc.sync.dma_start(
                            out=wt[:], in_=wfc0[c0 : c0 + cw, m * 128 : (m + 1) * 128]
                        )
                        nc.tensor.matmul(
                            hp[:], wt[:cw, :], mt[c][:cw, :], start=(c == 0), stop=(c == 2)
                        )
                    bt = sb.tile([128, 1], F32, name="b0t", tag="b0t")
                    nc.sync.dma_start(out=bt[:], in_=bfc0[m * 128 : (m + 1) * 128, :])
                    ht = sb.tile([128, MK], BF16, name=f"h0_{m}")
                    nc.scalar.activation(
                        ht[:], hp[:], mybir.ActivationFunctionType.Relu, bias=bt[:]
                    )
                    h0.append(ht)
                # h1T = relu(W_fc1^T @ h0T + b_fc1)
                h1 = []
                for m in range(4):
                    hp = ps.tile([128, MK], F32, name="h1p", tag="h1p")
                    for c in range(4):
                        wt = sb.tile([128, 128], BF16, name="w1t", tag="w1t")
                        nc.sync.dma_start(
                            out=wt[:],
                            in_=wfc1[c * 128 : (c + 1) * 128, m * 128 : (m + 1) * 128],
                        )
                        nc.tensor.matmul(
                            hp[:], wt[:], h1_src(h0, c), start=(c == 0), stop=(c == 3)
                        )
                    bt = sb.tile([128, 1], F32, name="b1t", tag="b1t")
                    nc.sync.dma_start(out=bt[:], in_=bfc1[m * 128 : (m + 1) * 128, :])
                    ht = sb.tile([128, MK], BF16, name=f"h1_{m}")
                    nc.scalar.activation(
                        ht[:], hp[:], mybir.ActivationFunctionType.Relu, bias=bt[:]
                    )
                    h1.append(ht)
                # out = W_last^T @ h1T + b_last
                op = ps.tile([1, MK], F32, name="op", tag="op")
                for c in range(4):
                    wt = sb.tile([128, 1], BF16, name="wlt", tag="wlt")
                    nc.sync.dma_start(out=wt[:], in_=wlast[c * 128 : (c + 1) * 128, :])
                    nc.tensor.matmul(
                        op[:], wt[:], h1[c][:], start=(c == 0), stop=(c == 3)
                    )
                blt = sb.tile([1, 1], F32, name="blt")
                nc.sync.dma_start(out=blt[:], in_=blast[:])
                outs = sb.tile([1, MK], F32, name="outs")
                nc.vector.tensor_tensor(
                    out=outs[:], in0=op[:], in1=blt[:].to_broadcast([1, MK]),
                    op=mybir.AluOpType.add,
                )
                nc.sync.dma_start(out=out_ext[:], in_=outs[:])

    nc.compile()
    return nc


def h1_src(h0, c):
    return h0[c][:]


def _prep_inputs(inputs):
    """Host-side sharding/preprocessing. Index-only work plus dtype casts."""
    f_atoms = np.asarray(inputs["f_atoms"], np.float32)
    f_bonds = np.asarray(inputs["f_bonds"], np.float32)
    a2b = np.asarray(inputs["a2b"], np.int64)
    b2a = np.asarray(inputs["b2a"], np.int64)
    b2revb = np.asarray(inputs["b2revb"], np.int64)

    # map global bond id -> Z row (core k slice is padded to BKP rows)
    def zrow(idx):
        return ((idx // BK) * BKP + (idx % BK)).astype(np.int32)

    nbr = a2b[b2a]  # [B, 6] bond ids feeding each bond's source atom
    it_idx_g = np.concatenate([zrow(nbr), zrow(b2revb)[:, None]], axis=1)  # [B,7]
    fin_idx_g = zrow(a2b)  # [A, 6]

    w = {}
    w["wi"] = np.asarray(inputs["W_i"], np.float32).astype(bf16)
    w["wh"] = np.asarray(inputs["W_h"], np.float32).astype(bf16)
    W_o = np.asarray(inputs["W_o"], np.float32)
    w["wo1"] = W_o[:ATOM_F].astype(bf16)
    w["wo2"] = W_o[ATOM_F:].astype(bf16)
    w["wfc0"] = np.asarray(inputs["W_fc0"], np.float32).astype(bf16)
    w["wfc1"] = np.asarray(inputs["W_fc1"], np.float32).astype(bf16)
    w["wlast"] = np.asarray(inputs["W_last"], np.float32).astype(bf16)
    w["bi_r"] = np.tile(np.asarray(inputs["b_i"], np.float32)[None, :], (128, 1))
    w["bh_r"] = np.tile(np.asarray(inputs["b_h"], np.float32)[None, :], (128, 1))
    w["bo_r"] = np.tile(np.asarray(inputs["b_o"], np.float32)[None, :], (128, 1))
    w["bfc0"] = np.asarray(inputs["b_fc0"], np.float32).reshape(RO, 1)
    w["bfc1"] = np.asarray(inputs["b_fc1"], np.float32).reshape(RO, 1)
    w["blast"] = np.asarray(inputs["b_last"], np.float32).reshape(1, 1)
    psel = np.zeros((125, 5), np.float32)
    psel[np.arange(125), np.arange(125) // APM] = 1.0
    w["psel_in"] = psel.astype(bf16)

    in_maps = []
    for k in range(NCORES):
        bs, be = k * BK, (k + 1) * BK
        as_, ae = k * AK, (k + 1) * AK
        fbt = np.zeros((BOND_F, BKP), np.float32)
        fbt[:, :BK] = f_bonds[bs:be].T
        fat = f_atoms[as_:ae].T.copy()
        iti = np.zeros((BKP, 7), np.int32)
        iti[:BK] = it_idx_g[bs:be]
        m = dict(w)
        m["fbT"] = fbt.astype(bf16).reshape(BOND_F, BT, 128)
        m["faT"] = fat.astype(bf16).reshape(ATOM_F, AT, 125)
        m["it_idx"] = iti.reshape(BT, 128, 7)
        m["fin_idx"] = fin_idx_g[as_:ae].astype(np.int32).reshape(AT, 125, 6)
        in_maps.append(m)
    return in_maps


def kernel(**inputs) -> np.ndarray:
    global _NC_CACHE
    if _NC_CACHE is None:
        _NC_CACHE = build()
    nc = _NC_CACHE
    in_maps = _prep_inputs(inputs)
    res = run_bass_kernel_spmd(nc, in_maps, core_ids=list(range(NCORES)))
    out = np.concatenate(
        [res.results[k]["out"].reshape(-1) for k in range(NCORES)], axis=0
    )
    return out.astype(np.float32)
